# revision 1
# baseline (speedup 1.0000x reference)
"""Trainium2 Bass kernel for nn_Attention_11991548690893.

Reference semantics (faithfully-reproduced bug): q = k = v = the key
projection, so only the middle third of W_attn is used and the attention
matrix S = kh @ kh^T is SYMMETRIC.  We exploit:
  - Megatron head-sharding: core c owns heads 2c, 2c+1 (128 head-dims),
    computes a partial output against its 128 rows of W_proj; the host
    sums the 8 partials and adds b_proj.
  - Host-side transpose of x -> xT so the on-device k projection needs
    no transposes of the big activation.
  - Symmetry of S: exp(S) row-blocks serve directly as [k-part, q-free]
    operands for the second attention matmul (no transpose).
  - Softmax without max-subtraction (logits are bounded ~15 here; fp32
    exp is safe) with rowsum from the ACT accum_out port.
  - fp32r matmuls (single-pass fp32 on the PE, 1 cycle/row at N>=512).
"""

import numpy as np

import concourse.bass as bass
import concourse.mybir as mybir
import concourse.tile as tile
from concourse import bacc
from concourse.bass_utils import run_bass_kernel_spmd

F32 = mybir.dt.float32
F32R = mybir.dt.float32r

B = 2
L = 2048
D = 1024
H = 16
DH = 64
NCORES = 8
DHC = 128            # head-dims per core (2 heads x 64)
L2 = B * L           # 4096
P = 128
NBLK = L // P        # 16 l-blocks per batch
SCALE = 1.0 / np.sqrt(DH)   # 0.125


def _build_kernel(ctx, tc, xT, wk, bk, wp, ident_dram, out):
    nc = tc.nc

    singles = ctx.enter_context(tc.tile_pool(name="singles", bufs=1))
    xpool = ctx.enter_context(tc.tile_pool(name="xpool", bufs=2))
    spool = ctx.enter_context(tc.tile_pool(name="spool", bufs=4))
    rpool = ctx.enter_context(tc.tile_pool(name="rpool", bufs=2))
    otpool = ctx.enter_context(tc.tile_pool(name="otpool", bufs=1))
    opool = ctx.enter_context(tc.tile_pool(name="opool", bufs=3))
    ps_main = ctx.enter_context(tc.tile_pool(name="ps_main", bufs=2, space="PSUM"))
    ps_out = ctx.enter_context(tc.tile_pool(name="ps_out", bufs=1, space="PSUM"))
    dpool = ctx.enter_context(tc.tile_pool(name="dpool", bufs=2, space="DRAM"))

    ident = singles.tile([P, P], F32R)
    nc.sync.dma_start(ident, ident_dram)

    wk_sb = singles.tile([P, 8, DHC], F32R)   # W_k slice, D-major tiles
    nc.sync.dma_start(wk_sb, wk.rearrange("(o p) m -> p o m", p=P))
    bk_sb = singles.tile([P, 1], F32)
    nc.sync.dma_start(bk_sb, bk)
    wp_sb = singles.tile([DH, 2, D], F32R)   # W_proj rows split per head
    nc.sync.dma_start(wp_sb, wp.rearrange("(t p) d -> p t d", p=DH))

    # ---- Phase 1: kT chunks = (x @ Wk + bk)^T, [128 dh, 512 l] x 8 ----
    xTr = xT.rearrange("(o p) l -> p o l", p=P)   # [128, 8, 4096]
    kT = []                                       # 8 x [128, 512]
    for lc in range(8):
        xc = xpool.tile([P, 8, 512], F32R, tag="xc")
        nc.sync.dma_start(xc, xTr[:, :, lc * 512:(lc + 1) * 512])
        ps = ps_main.tile([P, 512], F32, tag="mm")
        for dc in range(8):
            nc.tensor.matmul(
                ps,
                wk_sb[:, dc],
                xc[:, dc],
                start=(dc == 0),
                stop=(dc == 7),
            )
        kt = singles.tile([P, 512], F32R, tag=f"kt{lc}")
        nc.vector.tensor_scalar_add(kt, ps, bk_sb)
        kT.append(kt)

    # ---- k natural blocks + ones cols: knat [128 l, 32 blk, 130] ----
    # per block: [0:64]=head A, 64=ones, [65:129]=head B, 129=ones, so
    # [:, i, 65*h2 : 65*h2+65] is [kh_block | 1] — the out^T stationary
    # whose last column accumulates the softmax denominators.
    knat = singles.tile([P, 32, 2, P], F32R)
    ones32 = singles.tile([P, 32], F32)
    nc.vector.memset(ones32, 1.0)
    nc.vector.tensor_copy(knat[:, :, 0, 64:65], ones32.unsqueeze(-1))
    nc.vector.tensor_copy(knat[:, :, 1, 64:65], ones32.unsqueeze(-1))
    zpad = singles.tile([P, 2, 63], F32)
    nc.vector.memset(zpad, 0.0)
    for i in range(32):
        nc.vector.tensor_copy(knat[:, i, :, 65:], zpad)
    for i in range(32):
        tps = ps_main.tile([P, P], F32R, tag="mm")
        nc.tensor.transpose(tps, kT[i // 4][:, (i % 4) * P:(i % 4 + 1) * P], ident)
        nc.vector.tensor_copy(knat[:, i, 0, 0:64], tps[:, 0:64])
        nc.vector.tensor_copy(knat[:, i, 1, 0:64], tps[:, 64:128])

    def khT_chunk(b_, h2, c512):
        """[64, 512] slice of kT for batch b_, in-core head h2, l-chunk c512."""
        t = kT[b_ * 4 + c512]
        return t[h2 * DH:(h2 + 1) * DH, :]

    # ---- Phase 2: attention per batch, 2 heads; out^T accumulated in PSUM ----
    for b_ in range(B):
        oT_sb = []
        for h2 in range(2):
            oT_ps = ps_out.tile([P, L], F32, tag="ot")   # rows 0:64 out^T, row 64 denom, 65+ pad

            def att_block(i):
                # stationary [64, 128]: q-block i of khT
                lhsT_att = kT[b_ * 4 + i // 4][
                    h2 * DH:(h2 + 1) * DH, (i % 4) * P:(i % 4 + 1) * P
                ]
                Sb = spool.tile([P, L], F32R, tag="S")    # raw exp(S) row-block
                for kc in range(2):
                    aps = ps_main.tile([P, 1024], F32, tag="mm")
                    for n2 in range(2):
                        nc.tensor.matmul(
                            aps[:, n2 * 512:(n2 + 1) * 512],
                            lhsT_att,
                            khT_chunk(b_, h2, kc * 2 + n2),
                            start=True,
                            stop=True,
                        )
                    nc.scalar.activation(
                        Sb[:, kc * 1024:(kc + 1) * 1024],
                        aps,
                        mybir.ActivationFunctionType.Exp,
                        scale=SCALE,
                    )
                return Sb

            def outT_block(i, Sb):
                # [out^T | denom] += [kh_blk | 1]^T @ expS_blk (S symmetric)
                lhsT_o = knat[:, b_ * NBLK + i, h2]
                for qc in range(4):
                    nc.tensor.matmul(
                        oT_ps[:, qc * 512:(qc + 1) * 512],
                        lhsT_o,
                        Sb[:, qc * 512:(qc + 1) * 512],
                        start=(i == 0),
                        stop=(i == NBLK - 1),
                        skip_group_check=True,
                    )

            # software pipeline: emit att(i+1) before outT(i) so the PE
            # never blocks on exp(i) — keeps the HAM clock warm.
            Sb_prev = att_block(0)
            for i in range(1, NBLK):
                Sb_cur = att_block(i)
                outT_block(i - 1, Sb_prev)
                Sb_prev = Sb_cur
            outT_block(NBLK - 1, Sb_prev)
            # normalize: out^T row-block / denom (broadcast along partitions)
            recip = rpool.tile([1, L], F32, tag="recip")
            nc.vector.reciprocal(recip, oT_ps[DH:DH + 1, :])
            rdram = dpool.tile([1, L], F32)
            nc.sync.dma_start(rdram, recip)
            bcast = otpool.tile([DH, L], F32, tag="bc")
            nc.sync.dma_start(
                bcast,
                bass.AP(tensor=rdram.tensor, offset=rdram.offset,
                        ap=[[0, DH]] + list(rdram.ap)[1:]),
            )
            osb_h = otpool.tile([DH, L], F32R, tag=f"oT{h2}")
            nc.vector.tensor_mul(osb_h, oT_ps[0:DH, :], bcast)
            oT_sb.append(osb_h)
        # ---- Phase 3: partial = sum_h out_h^T.T @ Wp_h (two K=64 matmuls) ----
        for qt in range(NBLK):
            pps = ps_main.tile([P, 1024], F32, tag="mm")
            for n2 in range(2):
                for h2 in range(2):
                    nc.tensor.matmul(
                        pps[:, n2 * 512:(n2 + 1) * 512],
                        oT_sb[h2][:, qt * P:(qt + 1) * P],
                        wp_sb[:, h2, n2 * 512:(n2 + 1) * 512],
                        start=(h2 == 0),
                        stop=(h2 == 1),
                    )
            osb = opool.tile([P, D], F32, tag="osb")
            nc.vector.tensor_copy(osb, pps)
            nc.sync.dma_start(out[b_ * L + qt * P: b_ * L + (qt + 1) * P, :], osb)


_NC_CACHE = None


def _get_nc():
    global _NC_CACHE
    if _NC_CACHE is None:
        nc = bacc.Bacc("TRN2", target_bir_lowering=False)
        xT = nc.dram_tensor("xt", [D, L2], F32R, kind="ExternalInput").ap()
        wk = nc.dram_tensor("wk", [D, DHC], F32R, kind="ExternalInput").ap()
        bk = nc.dram_tensor("bk", [DHC, 1], F32, kind="ExternalInput").ap()
        wp = nc.dram_tensor("wp", [DHC, D], F32R, kind="ExternalInput").ap()
        ident = nc.dram_tensor("ident", [P, P], F32R, kind="ExternalInput").ap()
        out = nc.dram_tensor("out", [L2, D], F32, kind="ExternalOutput").ap()
        from contextlib import ExitStack
        with tile.TileContext(nc) as tc, ExitStack() as ctx:
            _build_kernel(ctx, tc, xT, wk, bk, wp, ident, out)
        nc.compile()
        _NC_CACHE = nc
    return _NC_CACHE


def _run(inputs, trace=False):
    x = np.asarray(inputs["x"], dtype=np.float32)
    W_attn = np.asarray(inputs["W_attn"], dtype=np.float32)
    b_attn = np.asarray(inputs["b_attn"], dtype=np.float32)
    W_proj = np.asarray(inputs["W_proj"], dtype=np.float32)
    b_proj = np.asarray(inputs["b_proj"], dtype=np.float32)

    xT = np.ascontiguousarray(x.reshape(L2, D).T)           # [1024, 4096]
    Wk = W_attn[:, D:2 * D]                                  # [1024, 1024]
    bk = b_attn[D:2 * D]                                     # [1024]

    in_maps = []
    for c in range(NCORES):
        sl = slice(c * DHC, (c + 1) * DHC)
        in_maps.append({
            "xt": xT,
            "wk": np.ascontiguousarray(Wk[:, sl]),
            "bk": np.ascontiguousarray(bk[sl]).reshape(DHC, 1),
            "wp": np.ascontiguousarray(W_proj[sl, :]),
            "ident": np.eye(P, dtype=np.float32),
        })

    nc = _get_nc()
    res = run_bass_kernel_spmd(nc, in_maps, core_ids=list(range(NCORES)),
                               trace=trace)
    acc = res.results[0]["out"].astype(np.float64)
    for r in res.results[1:]:
        acc += r["out"]
    acc += b_proj
    return acc.astype(np.float32).reshape(B, L, D), res


def kernel(**inputs):
    out, _ = _run(inputs, trace=False)
    return out


def kernel_traced(**inputs):
    return _run(inputs, trace=True)



# revision 5
# speedup vs baseline: 1.4683x; 1.4683x over previous
"""Trainium2 Bass kernel for nn_Attention_11991548690893.

Reference semantics (faithfully-reproduced bug): q = k = v = the key
projection, so only the middle third of W_attn is used and the attention
matrix S = kh @ kh^T is SYMMETRIC.  Design:
  - Megatron head-sharding: core c owns heads 2c, 2c+1 (128 head-dims),
    computes a partial output against its 128 rows of W_proj; the host
    sums the 8 bf16 partials and adds b_proj.
  - bf16 matmul operands everywhere (PSUM accumulation stays fp32);
    rel-err gate is 2e-2, bf16 lands ~4e-3.
  - Symmetry: only the upper-triangular blocks of S are matmul'd and
    exp'd (136 of 256 per batch-head). The mirrored lower blocks of
    exp(S) are produced by xbar DMA transposes (SBUF->SBUF, off the
    PE/ACT critical engines). This rebalances ACT (exp) from ~128us to
    ~77us so the PE stream stays dense and the HAM clock stays warm.
  - k natural-layout blocks (outT stationary) built by DMA transpose
    of kT; the ones column accumulates softmax denominators in PSUM
    partition 64 during the second attention matmul.
  - Normalization: single-partition denom row -> fast-approx
    reciprocal -> gpsimd partition_broadcast -> one DVE multiply.
    No DRAM bounce, no slow iterative divide.
"""

import numpy as np
import ml_dtypes

import concourse.bass as bass
import concourse.mybir as mybir
import concourse.tile as tile
from concourse import bacc
from concourse.bass_utils import run_bass_kernel_spmd

F32 = mybir.dt.float32
BF16 = mybir.dt.bfloat16
EXP = mybir.ActivationFunctionType.Exp

B = 2
L = 2048
D = 1024
H = 16
DH = 64
NCORES = 8
DHC = 128            # head-dims per core (2 heads x 64)
L2 = B * L           # 4096
P = 128
NBLK = L // P        # 16 l-blocks per batch
SCALE = 1.0 / np.sqrt(DH)   # 0.125


def _build_kernel(ctx, tc, xT, wk, bk, wp, out):
    nc = tc.nc

    singles = ctx.enter_context(tc.tile_pool(name="singles", bufs=1))
    xpool = ctx.enter_context(tc.tile_pool(name="xpool", bufs=2))
    otpool = ctx.enter_context(tc.tile_pool(name="otpool", bufs=6))
    npool = ctx.enter_context(tc.tile_pool(name="npool", bufs=2))
    ps_mm = ctx.enter_context(tc.tile_pool(name="ps_mm", bufs=2, space="PSUM"))
    ps_ot = ctx.enter_context(tc.tile_pool(name="ps_ot", bufs=2, space="PSUM"))

    wk_sb = singles.tile([P, 8, P], BF16)     # W_k slice, D-major tiles
    nc.scalar.dma_start(wk_sb, wk.rearrange("(o p) m -> p o m", p=P))
    bk_sb = singles.tile([P, 1], F32)
    nc.scalar.dma_start(bk_sb, bk)
    wp_sb = singles.tile([DH, 2, D], BF16)    # W_proj rows split per head
    nc.scalar.dma_start(wp_sb, wp.rearrange("(t p) d -> p t d", p=DH))

    kT = singles.tile([P, B, L], BF16)        # [128 dh, batch, tok]
    # knat: [tok, blk(b*16+m), h2, {64 kh cols | ones | pad}]
    knat = singles.tile([P, 2 * NBLK, 2, 66], BF16)
    # whole-tile memset to 1.0 (contiguous): col 64 becomes the ones column
    # for the denominator row; cols 0:64 are overwritten by the transposes.
    nc.vector.memset(knat.rearrange("p a b c -> p (a b c)"), 1.0)
    strips = singles.tile([P, NBLK, L], BF16)  # exp(S) k-strips, q-free
    osb = singles.tile([P, NBLK, D], BF16)     # phase-3 output staging

    # ---- Phase 1: kT = (x @ Wk + bk)^T in [128, 1024] chunks ----
    xTr = xT.rearrange("(o p) l -> p o l", p=P)   # [128, 8, 4096]
    for lc in range(4):
        xc = xpool.tile([P, 8, 1024], BF16, tag="xc")
        nc.scalar.dma_start(xc, xTr[:, :, lc * 1024:(lc + 1) * 1024])
        aps = ps_mm.tile([P, 1024], F32, tag="mm")
        for n2 in range(2):
            for dc in range(8):
                nc.tensor.matmul(
                    aps[:, n2 * 512:(n2 + 1) * 512],
                    wk_sb[:, dc],
                    xc[:, dc, n2 * 512:(n2 + 1) * 512],
                    start=(dc == 0),
                    stop=(dc == 7),
                )
        nc.vector.tensor_scalar_add(
            kT[:, lc // 2, (lc % 2) * 1024:(lc % 2) * 1024 + 1024], aps, bk_sb)

    # knat via DMA transpose of kT (chunk-major rows land as [tok%128, blk]).
    # The xbar ignores sub-4KB mid-dim strides on the destination, so
    # transpose into a contiguous staging tile and DVE-copy into the
    # 66-wide assembled layout.
    knd = singles.tile([P, B, 2, NBLK, 64], BF16)
    for b_ in range(B):
        for h2 in range(2):
            nc.sync.dma_start_transpose(
                knd[:, b_, h2],
                kT[h2 * DH:(h2 + 1) * DH, b_, :],
            )
            nc.vector.tensor_copy(
                knat[:, b_ * NBLK:(b_ + 1) * NBLK, h2, 0:64], knd[:, b_, h2])

    # ---- Phase 2: attention per (batch, head-pair); S upper-tri only ----
    oth_tiles = {}

    def do_bh(b_, h2):
        khT = kT[h2 * DH:(h2 + 1) * DH, b_, :]   # [64, 2048]
        ot0 = ps_ot.tile([DH + 1, 1024], F32, tag="ot")
        ot1 = ps_ot.tile([DH + 1, 1024], F32, tag="ot")
        ot = [ot0, ot1]

        def att(m):
            off = m * P
            span = L - off
            stat = khT[:, off:off + P]
            for c0 in range(0, span, 1024):
                cw = min(1024, span - c0)
                aps = ps_mm.tile([P, 1024], F32, tag="mm")
                for n0 in range(0, cw, 512):
                    w = min(512, cw - n0)
                    nc.tensor.matmul(
                        aps[:, n0:n0 + w],
                        stat,
                        khT[:, off + c0 + n0:off + c0 + n0 + w],
                        start=True,
                        stop=True,
                    )
                nc.scalar.activation(
                    strips[:, m, off + c0:off + c0 + cw], aps[:, 0:cw],
                    EXP, scale=SCALE)
            if m < NBLK - 1:
                # mirror exp'd blocks (m, n>m) into later strips via xbar
                nc.sync.dma_start_transpose(
                    strips[:, m + 1:NBLK, off:off + P],
                    strips[:, m, off + P:L],
                )

        def outT(m):
            lhsT = knat[:, b_ * NBLK + m, h2, 0:65]
            for half in range(2):
                for n0 in (0, 512):
                    nc.tensor.matmul(
                        ot[half][:, n0:n0 + 512],
                        lhsT,
                        strips[:, m, half * 1024 + n0:half * 1024 + n0 + 512],
                        start=(m == 0),
                        stop=(m == NBLK - 1),
                        skip_group_check=True,
                    )

        att(0)
        att(1)
        for m in range(2, NBLK):
            outT(m - 2)
            att(m)
        outT(NBLK - 2)
        outT(NBLK - 1)

        for half in range(2):
            nrow = npool.tile([1, 1024], F32, tag="nrow")
            nc.vector.tensor_copy(nrow, ot[half][DH:DH + 1, :])
            rec = npool.tile([1, 1024], F32, tag="rec")
            nc.vector.reciprocal_approx_fast(rec, nrow)
            bc = npool.tile([DH, 1024], F32, tag="bc")
            nc.gpsimd.partition_broadcast(bc, rec)
            oth = otpool.tile([DH, 1024], BF16, tag="oth")
            nc.vector.tensor_mul(oth, ot[half][0:DH, :], bc)
            oth_tiles[(b_, h2, half)] = oth

    # ---- Phase 3: partial = sum_h out_h^T.T @ Wp_h per batch ----
    def ph3(b_):
        for qt in range(NBLK):
            pps = ps_mm.tile([P, 1024], F32, tag="mm")
            oths = [oth_tiles[(b_, h2, qt // 8)] for h2 in range(2)]
            for n2 in range(2):
                for h2 in range(2):
                    nc.tensor.matmul(
                        pps[:, n2 * 512:(n2 + 1) * 512],
                        oths[h2][:, (qt % 8) * P:(qt % 8) * P + P],
                        wp_sb[:, h2, n2 * 512:(n2 + 1) * 512],
                        start=(h2 == 0),
                        stop=(h2 == 1),
                    )
            nc.vector.tensor_copy(osb[:, qt], pps)
        nc.scalar.dma_start(
            out[b_ * L:(b_ + 1) * L, :].rearrange("(q p) d -> p q d", p=P),
            osb,
        )

    for b_ in range(B):
        do_bh(b_, 0)
        do_bh(b_, 1)
        ph3(b_)


_NC_CACHE = None


def _get_nc():
    global _NC_CACHE
    if _NC_CACHE is None:
        nc = bacc.Bacc("TRN2", target_bir_lowering=False)
        xT = nc.dram_tensor("xt", [D, L2], BF16, kind="ExternalInput").ap()
        wk = nc.dram_tensor("wk", [D, DHC], BF16, kind="ExternalInput").ap()
        bk = nc.dram_tensor("bk", [DHC, 1], F32, kind="ExternalInput").ap()
        wp = nc.dram_tensor("wp", [DHC, D], BF16, kind="ExternalInput").ap()
        out = nc.dram_tensor("out", [L2, D], BF16, kind="ExternalOutput").ap()
        from contextlib import ExitStack
        with tile.TileContext(nc) as tc, ExitStack() as ctx:
            _build_kernel(ctx, tc, xT, wk, bk, wp, out)
        nc.compile()
        _NC_CACHE = nc
    return _NC_CACHE


def _run(inputs, trace=False):
    x = np.asarray(inputs["x"], dtype=np.float32)
    W_attn = np.asarray(inputs["W_attn"], dtype=np.float32)
    b_attn = np.asarray(inputs["b_attn"], dtype=np.float32)
    W_proj = np.asarray(inputs["W_proj"], dtype=np.float32)
    b_proj = np.asarray(inputs["b_proj"], dtype=np.float32)

    bf16 = ml_dtypes.bfloat16
    xT = x.reshape(L2, D).T.astype(bf16)                     # [1024, 4096]
    Wk = W_attn[:, D:2 * D]                                  # [1024, 1024]
    bk = b_attn[D:2 * D]                                     # [1024]

    in_maps = []
    for c in range(NCORES):
        sl = slice(c * DHC, (c + 1) * DHC)
        in_maps.append({
            "xt": xT,
            "wk": Wk[:, sl].astype(bf16),
            "bk": np.ascontiguousarray(bk[sl]).reshape(DHC, 1),
            "wp": W_proj[sl, :].astype(bf16),
        })

    nc = _get_nc()
    res = run_bass_kernel_spmd(nc, in_maps, core_ids=list(range(NCORES)),
                               trace=trace)
    acc = np.zeros((L2, D), dtype=np.float32)
    for r in res.results:
        acc += r["out"].astype(np.float32)
    acc += b_proj
    return acc.reshape(B, L, D), res


def kernel(**inputs):
    out, _ = _run(inputs, trace=False)
    return out


def kernel_traced(**inputs):
    return _run(inputs, trace=True)


# revision 8
# speedup vs baseline: 1.5465x; 1.0533x over previous
"""Trainium2 Bass kernel for nn_Attention_11991548690893.

Reference semantics (faithfully-reproduced bug): q = k = v = the key
projection, so only the middle third of W_attn is used and the attention
matrix S = kh @ kh^T is SYMMETRIC.  Design:
  - Megatron head-sharding: core c owns heads 2c, 2c+1 (128 head-dims),
    computes a partial output against its 128 rows of W_proj; the host
    sums the 8 bf16 partials and adds b_proj.
  - bf16 matmul operands everywhere (PSUM accumulation stays fp32);
    rel-err gate is 2e-2, bf16 lands ~4e-3.
  - Symmetry: only the upper-triangular blocks of S are matmul'd and
    exp'd (136 of 256 per batch-head). The mirrored lower blocks of
    exp(S) are produced by xbar DMA transposes (SBUF->SBUF, off the
    PE/ACT critical engines). This rebalances ACT (exp) from ~128us to
    ~77us so the PE stream stays dense and the HAM clock stays warm.
  - k natural-layout blocks (outT stationary) built by DMA transpose
    of kT; the ones column accumulates softmax denominators in PSUM
    partition 64 during the second attention matmul.
  - Normalization: single-partition denom row -> fast-approx
    reciprocal -> gpsimd partition_broadcast -> one DVE multiply.
    No DRAM bounce, no slow iterative divide.
"""

import numpy as np
import ml_dtypes

import concourse.bass as bass
import concourse.mybir as mybir
import concourse.tile as tile
from concourse import bacc
from concourse.bass_utils import run_bass_kernel_spmd

F32 = mybir.dt.float32
BF16 = mybir.dt.bfloat16
EXP = mybir.ActivationFunctionType.Exp

B = 2
L = 2048
D = 1024
H = 16
DH = 64
NCORES = 8
DHC = 128            # head-dims per core (2 heads x 64)
L2 = B * L           # 4096
P = 128
NBLK = L // P        # 16 l-blocks per batch
SCALE = 1.0 / np.sqrt(DH)   # 0.125


def _build_kernel(ctx, tc, xT, wk, bk, wp, out):
    nc = tc.nc

    singles = ctx.enter_context(tc.tile_pool(name="singles", bufs=1))
    xpool = ctx.enter_context(tc.tile_pool(name="xpool", bufs=2))
    otpool = ctx.enter_context(tc.tile_pool(name="otpool", bufs=6))
    npool = ctx.enter_context(tc.tile_pool(name="npool", bufs=2))
    ps_mm = ctx.enter_context(tc.tile_pool(name="ps_mm", bufs=3, space="PSUM"))
    ps_ot = ctx.enter_context(tc.tile_pool(name="ps_ot", bufs=1, space="PSUM"))

    wk_sb = singles.tile([P, 8, P], BF16)     # W_k slice, D-major tiles
    nc.scalar.dma_start(wk_sb, wk.rearrange("(o p) m -> p o m", p=P))
    bk_sb = singles.tile([P, 1], F32)
    nc.scalar.dma_start(bk_sb, bk)
    wp_sb = singles.tile([DH, 2, D], BF16)    # W_proj rows split per head
    nc.scalar.dma_start(wp_sb, wp.rearrange("(t p) d -> p t d", p=DH))

    kT = singles.tile([P, B, L], BF16)        # [128 dh, batch, tok]
    # knat: [tok, blk(b*16+m), h2, {64 kh cols | ones | pad}]
    knat = singles.tile([P, 2 * NBLK, 2, 66], BF16)
    # whole-tile memset to 1.0 (contiguous): col 64 becomes the ones column
    # for the denominator row; cols 0:64 are overwritten by the transposes.
    nc.vector.memset(knat.rearrange("p a b c -> p (a b c)"), 1.0)
    strips = singles.tile([P, NBLK, L], BF16)  # exp(S) k-strips, q-free
    osb = singles.tile([P, NBLK, D], BF16)     # phase-3 output staging

    # ---- Phase 1: kT = (x @ Wk + bk)^T in [128, 1024] chunks ----
    xTr = xT.rearrange("(o p) l -> p o l", p=P)   # [128, 8, 4096]
    for lc in range(4):
        xc = xpool.tile([P, 8, 1024], BF16, tag="xc")
        nc.scalar.dma_start(xc, xTr[:, :, lc * 1024:(lc + 1) * 1024])
        aps = ps_mm.tile([P, 1024], F32, tag="mm")
        for n2 in range(2):
            for dc in range(8):
                nc.tensor.matmul(
                    aps[:, n2 * 512:(n2 + 1) * 512],
                    wk_sb[:, dc],
                    xc[:, dc, n2 * 512:(n2 + 1) * 512],
                    start=(dc == 0),
                    stop=(dc == 7),
                )
        nc.vector.tensor_scalar_add(
            kT[:, lc // 2, (lc % 2) * 1024:(lc % 2) * 1024 + 1024], aps, bk_sb)

    # knat via DMA transpose of kT (chunk-major rows land as [tok%128, blk]).
    # The xbar ignores sub-4KB mid-dim strides on the destination, so
    # transpose into a contiguous staging tile and DVE-copy into the
    # 66-wide assembled layout.
    knd = singles.tile([P, B, 2, NBLK, 64], BF16)
    for b_ in range(B):
        for h2 in range(2):
            nc.sync.dma_start_transpose(
                knd[:, b_, h2],
                kT[h2 * DH:(h2 + 1) * DH, b_, :],
            )
            nc.vector.tensor_copy(
                knat[:, b_ * NBLK:(b_ + 1) * NBLK, h2, 0:64], knd[:, b_, h2])

    # ---- Phase 2: attention per (batch, head-pair); S upper-tri only ----
    oth_tiles = {}

    def do_bh(b_, h2):
        khT = kT[h2 * DH:(h2 + 1) * DH, b_, :]   # [64, 2048]

        def att(m):
            off = m * P
            span = L - off
            stat = khT[:, off:off + P]
            for c0 in range(0, span, 1024):
                cw = min(1024, span - c0)
                aps = ps_mm.tile([P, 1024], F32, tag="mm")
                for n0 in range(0, cw, 512):
                    w = min(512, cw - n0)
                    nc.tensor.matmul(
                        aps[:, n0:n0 + w],
                        stat,
                        khT[:, off + c0 + n0:off + c0 + n0 + w],
                        start=True,
                        stop=True,
                    )
                nc.scalar.activation(
                    strips[:, m, off + c0:off + c0 + cw], aps[:, 0:cw],
                    EXP, scale=SCALE)
            if m < NBLK - 1:
                # mirror exp'd blocks (m, n>m) into later strips via xbar
                nc.sync.dma_start_transpose(
                    strips[:, m + 1:NBLK, off:off + P],
                    strips[:, m, off + P:L],
                )

        def outT(ot, half, m):
            lhsT = knat[:, b_ * NBLK + m, h2, 0:65]
            for n0 in (0, 512):
                nc.tensor.matmul(
                    ot[:, n0:n0 + 512],
                    lhsT,
                    strips[:, m, half * 1024 + n0:half * 1024 + n0 + 512],
                    start=(m == 0),
                    stop=(m == NBLK - 1),
                    skip_group_check=True,
                )

        def normalize(ot, half):
            # one full-tile copy frees the PSUM slot fast; chain runs on SBUF
            otc = npool.tile([DH + 1, 1024], F32, tag="otc")
            nc.vector.tensor_copy(otc, ot)
            nrow = npool.tile([1, 1024], F32, tag="nrow")
            nc.vector.tensor_copy(nrow, otc[DH:DH + 1, :])
            rec = npool.tile([1, 1024], F32, tag="rec")
            nc.vector.reciprocal_approx_fast(rec, nrow)
            bc = npool.tile([DH, 1024], F32, tag="bc")
            nc.gpsimd.partition_broadcast(bc, rec)
            oth = otpool.tile([DH, 1024], BF16, tag="oth")
            nc.vector.tensor_mul(oth, otc[0:DH, :], bc)
            oth_tiles[(b_, h2, half)] = oth

        # q-half 0 accumulates inside the m-loop (outT lags att by 4 so the
        # exp + mirror pipeline never stalls the PE); q-half 1 runs as a
        # dense post-loop PE pass over the persisted strips.
        ot0 = ps_ot.tile([DH + 1, 1024], F32, tag="ot")
        for m in range(4):
            att(m)
        for m in range(4, NBLK):
            outT(ot0, 0, m - 4)
            att(m)
        for m in range(NBLK - 4, NBLK):
            outT(ot0, 0, m)
        normalize(ot0, 0)
        ot1 = ps_ot.tile([DH + 1, 1024], F32, tag="ot")
        for m in range(NBLK):
            outT(ot1, 1, m)
        normalize(ot1, 1)

    # ---- Phase 3: partial = sum_h out_h^T.T @ Wp_h per batch ----
    def ph3(b_):
        for qt in range(NBLK):
            pps = ps_mm.tile([P, 1024], F32, tag="mm")
            oths = [oth_tiles[(b_, h2, qt // 8)] for h2 in range(2)]
            for n2 in range(2):
                for h2 in range(2):
                    nc.tensor.matmul(
                        pps[:, n2 * 512:(n2 + 1) * 512],
                        oths[h2][:, (qt % 8) * P:(qt % 8) * P + P],
                        wp_sb[:, h2, n2 * 512:(n2 + 1) * 512],
                        start=(h2 == 0),
                        stop=(h2 == 1),
                    )
            nc.vector.tensor_copy(osb[:, qt], pps)
        nc.scalar.dma_start(
            out[b_ * L:(b_ + 1) * L, :].rearrange("(q p) d -> p q d", p=P),
            osb,
        )

    for b_ in range(B):
        do_bh(b_, 0)
        do_bh(b_, 1)
        ph3(b_)


_NC_CACHE = None


def _get_nc():
    global _NC_CACHE
    if _NC_CACHE is None:
        nc = bacc.Bacc("TRN2", target_bir_lowering=False)
        xT = nc.dram_tensor("xt", [D, L2], BF16, kind="ExternalInput").ap()
        wk = nc.dram_tensor("wk", [D, DHC], BF16, kind="ExternalInput").ap()
        bk = nc.dram_tensor("bk", [DHC, 1], F32, kind="ExternalInput").ap()
        wp = nc.dram_tensor("wp", [DHC, D], BF16, kind="ExternalInput").ap()
        out = nc.dram_tensor("out", [L2, D], BF16, kind="ExternalOutput").ap()
        from contextlib import ExitStack
        with tile.TileContext(nc) as tc, ExitStack() as ctx:
            _build_kernel(ctx, tc, xT, wk, bk, wp, out)
        nc.compile()
        _NC_CACHE = nc
    return _NC_CACHE


def _run(inputs, trace=False):
    x = np.asarray(inputs["x"], dtype=np.float32)
    W_attn = np.asarray(inputs["W_attn"], dtype=np.float32)
    b_attn = np.asarray(inputs["b_attn"], dtype=np.float32)
    W_proj = np.asarray(inputs["W_proj"], dtype=np.float32)
    b_proj = np.asarray(inputs["b_proj"], dtype=np.float32)

    bf16 = ml_dtypes.bfloat16
    xT = x.reshape(L2, D).T.astype(bf16)                     # [1024, 4096]
    Wk = W_attn[:, D:2 * D]                                  # [1024, 1024]
    bk = b_attn[D:2 * D]                                     # [1024]

    in_maps = []
    for c in range(NCORES):
        sl = slice(c * DHC, (c + 1) * DHC)
        in_maps.append({
            "xt": xT,
            "wk": Wk[:, sl].astype(bf16),
            "bk": np.ascontiguousarray(bk[sl]).reshape(DHC, 1),
            "wp": W_proj[sl, :].astype(bf16),
        })

    nc = _get_nc()
    res = run_bass_kernel_spmd(nc, in_maps, core_ids=list(range(NCORES)),
                               trace=trace)
    acc = np.zeros((L2, D), dtype=np.float32)
    for r in res.results:
        acc += r["out"].astype(np.float32)
    acc += b_proj
    return acc.reshape(B, L, D), res


def kernel(**inputs):
    out, _ = _run(inputs, trace=False)
    return out


def kernel_traced(**inputs):
    return _run(inputs, trace=True)


# revision 17
# speedup vs baseline: 1.7459x; 1.1289x over previous
"""Trainium2 Bass kernel for nn_Attention_11991548690893.

Reference semantics (faithfully-reproduced bug): q = k = v = the key
projection, so only the middle third of W_attn is used and the attention
matrix S = kh @ kh^T is SYMMETRIC.  Design:
  - Megatron head-sharding: core c owns heads 2c, 2c+1 (128 head-dims),
    computes a partial output against its 128 rows of W_proj; the host
    sums the 8 bf16 partials and adds b_proj.
  - bf16 matmul operands everywhere (PSUM accumulation stays fp32);
    rel-err gate is 2e-2, bf16 lands ~4e-3.
  - Symmetry: only the upper-triangular blocks of S are matmul'd and
    exp'd (136 of 256 per batch-head). The mirrored lower blocks of
    exp(S) are produced by xbar DMA transposes (SBUF->SBUF, off the
    PE/ACT critical engines). This rebalances ACT (exp) from ~128us to
    ~77us so the PE stream stays dense and the HAM clock stays warm.
  - k natural-layout blocks (outT stationary) built by DMA transpose
    of kT; the ones column accumulates softmax denominators in PSUM
    partition 64 during the second attention matmul.
  - Normalization: single-partition denom row -> fast-approx
    reciprocal -> gpsimd partition_broadcast -> one DVE multiply.
    No DRAM bounce, no slow iterative divide.
"""

import numpy as np
import ml_dtypes

import concourse.bass as bass
import concourse.mybir as mybir
import concourse.tile as tile
from concourse import bacc
from concourse.bass_utils import run_bass_kernel_spmd

F32 = mybir.dt.float32
BF16 = mybir.dt.bfloat16
EXP = mybir.ActivationFunctionType.Exp

B = 2
L = 2048
D = 1024
H = 16
DH = 64
NCORES = 8
DHC = 128            # head-dims per core (2 heads x 64)
L2 = B * L           # 4096
P = 128
NBLK = L // P        # 16 l-blocks per batch
SCALE = 1.0 / np.sqrt(DH)   # 0.125


def _build_kernel(ctx, tc, xT, wk, bk, wp, out):
    nc = tc.nc

    singles = ctx.enter_context(tc.tile_pool(name="singles", bufs=1))
    xpool = ctx.enter_context(tc.tile_pool(name="xpool", bufs=2))
    otpool = ctx.enter_context(tc.tile_pool(name="otpool", bufs=6))
    otfpool = ctx.enter_context(tc.tile_pool(name="otfpool", bufs=3))
    npool = ctx.enter_context(tc.tile_pool(name="npool", bufs=1))
    scratch = ctx.enter_context(tc.tile_pool(name="scratch", bufs=2))
    ps_mm = ctx.enter_context(tc.tile_pool(name="ps_mm", bufs=3, space="PSUM"))
    ps_ot = ctx.enter_context(tc.tile_pool(name="ps_ot", bufs=1, space="PSUM"))

    wk_sb = singles.tile([P, 8, P], BF16)     # W_k slice, D-major tiles
    nc.scalar.dma_start(wk_sb, wk.rearrange("(o p) m -> p o m", p=P))
    bk_sb = singles.tile([P, 1], F32)
    nc.scalar.dma_start(bk_sb, bk)
    wp_sb = singles.tile([P, D], BF16)        # W_proj rows, all 128 head-dims
    nc.scalar.dma_start(wp_sb, wp)

    kT = singles.tile([P, B, L], BF16)        # [128 dh, batch, tok]
    # zero-padded per-head copies of kT: K=128 stationaries keep the full
    # PE array active (K=64 stationaries leave HAM permanently throttled)
    kTz = singles.tile([P, 2, B, L], BF16)    # [128, h2, batch, tok]
    nc.vector.memset(kTz.rearrange("p h b l -> p (h b l)"), 0.0)
    # knat: [tok, blk(b*16+m), h2, {64 kh cols | ones | pad}]
    knat = singles.tile([P, 2 * NBLK, 2, 66], BF16)
    # whole-tile memset to 1.0 (contiguous): col 64 becomes the ones column
    # for the denominator row; cols 0:64 are overwritten by the transposes.
    nc.vector.memset(knat.rearrange("p a b c -> p (a b c)"), 1.0)
    strips = singles.tile([P, NBLK, L], BF16)  # exp(S) k-strips, q-free
    osb = singles.tile([P, NBLK // 2, D], BF16)  # phase-3 staging (half batch)

    # ---- Phase 1: kT = (x @ Wk + bk)^T ----
    xTr = xT.rearrange("(o p) l -> p o l", p=P)   # [128, 8, 4096]
    aps = None
    for lc in range(8):
        xc = xpool.tile([P, 8, 512], BF16, tag="xc")
        nc.scalar.dma_start(xc, xTr[:, :, lc * 512:(lc + 1) * 512])
        if lc % 2 == 0:
            aps = ps_mm.tile([P, 1024], F32, tag="mm")
        for dc in range(8):
            nc.tensor.matmul(
                aps[:, (lc % 2) * 512:(lc % 2 + 1) * 512],
                wk_sb[:, dc],
                xc[:, dc],
                start=(dc == 0),
                stop=(dc == 7),
            )
        if lc % 2 == 1:
            nc.vector.tensor_scalar_add(
                kT[:, lc // 4, (lc // 2 % 2) * 1024:(lc // 2 % 2) * 1024 + 1024],
                aps, bk_sb)
    for b_ in range(B):
        for h2 in range(2):
            nc.vector.tensor_copy(
                kTz[h2 * DH:(h2 + 1) * DH, h2, b_, :],
                kT[h2 * DH:(h2 + 1) * DH, b_, :])

    # knat via DMA transpose of kT (chunk-major rows land as [tok%128, blk]).
    # The xbar ignores sub-4KB mid-dim strides on the destination, so
    # transpose into a contiguous staging tile and DVE-copy into the
    # 66-wide assembled layout.
    for b_ in range(B):
        for h2 in range(2):
            knd = scratch.tile([P, NBLK, 64], BF16, tag="knd")
            nc.sync.dma_start_transpose(
                knd,
                kT[h2 * DH:(h2 + 1) * DH, b_, :],
            )
            nc.vector.tensor_copy(
                knat[:, b_ * NBLK:(b_ + 1) * NBLK, h2, 0:64], knd)

    # ---- Phase 2: attention per (batch, head-pair); S upper-tri only ----
    oth_tiles = {}

    def do_bh(b_, h2):
        khT = kT[:, b_, :]                        # [128, 2048] both heads
        statz = kTz[:, h2, b_, :]                 # [128, 2048] head h2 + zeros

        def att(m):
            off = m * P
            span = L - off
            stat = statz[:, off:off + P]
            for c0 in range(0, span, 1024):
                cw = min(1024, span - c0)
                aps = ps_mm.tile([P, 1024], F32, tag="mm")
                for n0 in range(0, cw, 512):
                    w = min(512, cw - n0)
                    nc.tensor.matmul(
                        aps[:, n0:n0 + w],
                        stat,
                        khT[:, off + c0 + n0:off + c0 + n0 + w],
                        start=True,
                        stop=True,
                    )
                nc.scalar.activation(
                    strips[:, m, off + c0:off + c0 + cw], aps[:, 0:cw],
                    EXP, scale=SCALE)
            if m < NBLK - 1:
                # mirror exp'd blocks (m, n>m) into later strips via xbar
                nc.sync.dma_start_transpose(
                    strips[:, m + 1:NBLK, off:off + P],
                    strips[:, m, off + P:L],
                )

        def outT(ot, half, m):
            lhsT = knat[:, b_ * NBLK + m, h2, 0:65]
            for n0 in (0, 512):
                nc.tensor.matmul(
                    ot[:, n0:n0 + 512],
                    lhsT,
                    strips[:, m, half * 1024 + n0:half * 1024 + n0 + 512],
                    start=(m == 0),
                    stop=(m == NBLK - 1),
                    skip_group_check=True,
                )

        def normalize(ot, half):
            # one full-tile copy frees the PSUM slot fast; chain runs on SBUF
            otc = npool.tile([DH + 1, 1024], F32, tag="otc")
            nc.vector.tensor_copy(otc, ot)
            nrow = npool.tile([1, 1024], F32, tag="nrow")
            nc.vector.tensor_copy(nrow, otc[DH:DH + 1, :])
            rec = npool.tile([1, 1024], F32, tag="rec")
            nc.vector.reciprocal_approx_fast(rec, nrow)
            bc = npool.tile([DH, 1024], F32, tag="bc")
            nc.gpsimd.partition_broadcast(bc, rec)
            oth = otpool.tile([DH, 1024], BF16, tag="oth")
            nc.vector.tensor_mul(oth, otc[0:DH, :], bc)
            oth_tiles[(b_, h2, half)] = oth

        # q-half 0 accumulates inside the m-loop (outT lags att by 4 so the
        # exp + mirror pipeline never stalls the PE); q-half 1 runs as a
        # dense post-loop PE pass over the persisted strips.
        ot0 = ps_ot.tile([DH + 1, 1024], F32, tag="ot")
        for m in range(4):
            att(m)
        for m in range(4, NBLK):
            outT(ot0, 0, m - 4)
            att(m)
        for m in range(NBLK - 4, NBLK):
            outT(ot0, 0, m)
        normalize(ot0, 0)
        ot1 = ps_ot.tile([DH + 1, 1024], F32, tag="ot")
        for m in range(NBLK):
            outT(ot1, 1, m)
        normalize(ot1, 1)

    # ---- Phase 3: partial = [oth_h0; oth_h1]^T.T @ Wp (K=128) per batch ----
    def ph3(b_):
        for half in range(2):
            othf = otfpool.tile([P, 1024], BF16, tag="othf")
            nc.vector.tensor_copy(othf[0:DH, :], oth_tiles[(b_, 0, half)])
            nc.vector.tensor_copy(othf[DH:P, :], oth_tiles[(b_, 1, half)])
            for q8 in range(NBLK // 2):
                qt = half * 8 + q8
                pps = ps_mm.tile([P, 1024], F32, tag="mm")
                for n2 in range(2):
                    nc.tensor.matmul(
                        pps[:, n2 * 512:(n2 + 1) * 512],
                        othf[:, q8 * P:q8 * P + P],
                        wp_sb[:, n2 * 512:(n2 + 1) * 512],
                        start=True,
                        stop=True,
                    )
                nc.vector.tensor_copy(osb[:, q8], pps)
            nc.scalar.dma_start(
                out[b_ * L + half * (L // 2):b_ * L + (half + 1) * (L // 2), :]
                .rearrange("(q p) d -> p q d", p=P),
                osb,
            )

    for b_ in range(B):
        do_bh(b_, 0)
        do_bh(b_, 1)
        ph3(b_)


_NC_CACHE = None


def _get_nc():
    global _NC_CACHE
    if _NC_CACHE is None:
        nc = bacc.Bacc("TRN2", target_bir_lowering=False)
        xT = nc.dram_tensor("xt", [D, L2], BF16, kind="ExternalInput").ap()
        wk = nc.dram_tensor("wk", [D, DHC], BF16, kind="ExternalInput").ap()
        bk = nc.dram_tensor("bk", [DHC, 1], F32, kind="ExternalInput").ap()
        wp = nc.dram_tensor("wp", [DHC, D], BF16, kind="ExternalInput").ap()
        out = nc.dram_tensor("out", [L2, D], BF16, kind="ExternalOutput").ap()
        from contextlib import ExitStack
        with tile.TileContext(nc) as tc, ExitStack() as ctx:
            _build_kernel(ctx, tc, xT, wk, bk, wp, out)
        nc.compile()
        _NC_CACHE = nc
    return _NC_CACHE


def _run(inputs, trace=False):
    x = np.asarray(inputs["x"], dtype=np.float32)
    W_attn = np.asarray(inputs["W_attn"], dtype=np.float32)
    b_attn = np.asarray(inputs["b_attn"], dtype=np.float32)
    W_proj = np.asarray(inputs["W_proj"], dtype=np.float32)
    b_proj = np.asarray(inputs["b_proj"], dtype=np.float32)

    bf16 = ml_dtypes.bfloat16
    xT = x.reshape(L2, D).T.astype(bf16)                     # [1024, 4096]
    Wk = W_attn[:, D:2 * D]                                  # [1024, 1024]
    bk = b_attn[D:2 * D]                                     # [1024]

    in_maps = []
    for c in range(NCORES):
        sl = slice(c * DHC, (c + 1) * DHC)
        in_maps.append({
            "xt": xT,
            "wk": Wk[:, sl].astype(bf16),
            "bk": np.ascontiguousarray(bk[sl]).reshape(DHC, 1),
            "wp": W_proj[sl, :].astype(bf16),
        })

    nc = _get_nc()
    res = run_bass_kernel_spmd(nc, in_maps, core_ids=list(range(NCORES)),
                               trace=trace)
    acc = np.zeros((L2, D), dtype=np.float32)
    for r in res.results:
        acc += r["out"].astype(np.float32)
    acc += b_proj
    return acc.reshape(B, L, D), res


def kernel(**inputs):
    out, _ = _run(inputs, trace=False)
    return out


def kernel_traced(**inputs):
    return _run(inputs, trace=True)
